# revision 1
# baseline (speedup 1.0000x reference)
"""Trainium2 Bass kernel for the dual-softmax interaction (BiDAF-style) layer.

Math (per batch b, with E_q [q,H], E_p [p,H], W = [w_q, w_p, w_m]):
    U[p,q]  = sum_h (E_p*w_m)[p,h] E_q[q,h] + sp[p] + sq[q]   (+ mask bias)
    A_p     = softmax_q(U),  B_p = softmax_p(U)
    A1 = A_p @ E_q, B1 = B_p^T @ E_p, A2 = A_p @ B1, B2 = B_p^T @ A1
    G_q_p = [E_p, A1, A2, E_p*A1, E_p*A2];  G_p_q = [E_q, B1, B2, E_q*B1, E_q*B2]

Sharding: 8 cores = 4 batches x 2 "sides" (side 0 owns the p-indexed output
G_q_p, side 1 the q-indexed output G_p_q). Both sides run the SAME device
program on swapped operands, exploiting the symmetry U_B = U_A^T.

Device-side trick: the rank-1 terms sp/sq (and mask biases) factor out of the
softmaxes as per-row/column scalings delta=exp(sp_eff), gamma=exp(sq_eff).
They are folded on the host into scaled copies of the small [2048,256]
operands, so the device only computes S = exp(tri) with a clean K=256 matmul.
The exact transpose T = S^T is produced with the xbar DMA-transpose (bf16),
after which all attention products use natural layouts:
    Q1_num[j,h] = sum_i S[i,j] (delta*OWN)[i,h]   (lhsT = S chunks)
    [P1_num | sumO | P2_num] = one wide MM: lhsT = T chunks,
        rhs = [gamma*OTH (256) | gamma (1) | gamma-scaled Q1n (256)]
"""

import sys

if "/opt/trn_rl_repo" not in sys.path:
    sys.path.insert(0, "/opt/trn_rl_repo")

import ml_dtypes
import numpy as np

import concourse.bass as bass  # noqa: F401  (registers AP machinery)
import concourse.mybir as mybir
import concourse.tile as tile
from concourse import bacc
from concourse.bass_utils import run_bass_kernel_spmd

B, SEQ, H = 4, 2048, 256
P = 128
NS = SEQ // P  # 16 strips/blocks
RC = 260  # ownd row: 256 data + 1 sum-column + 3 pad
RW = 516  # comb row: 256 gamma*OTH + 1 gamma + 256 q1g + 3 pad
N_CORES = 8

f32 = mybir.dt.float32
bf16 = mybir.dt.bfloat16
bf16_np = ml_dtypes.bfloat16


def emit_kernel(nc, tc, own_f32, u_lhs, u_rhs, othg, ownd, g):
    Exp = mybir.ActivationFunctionType.Exp
    mult = mybir.AluOpType.mult

    with (
        tc.tile_pool(name="big", bufs=1) as big,
        tc.tile_pool(name="small", bufs=1) as small,
    ):
        S = big.tile([P, NS, SEQ], bf16)  # exp(tri), strip s on [:, s, :]
        T = big.tile([P, NS, SEQ], bf16)  # S^T, oth-strip k on [:, k, :]
        comb = big.tile([P, NS, RW], bf16)  # [gamma*OTH | gamma | q1g | pad]
        ownd_sb = big.tile([P, NS, RC], bf16)

        # ---- U phase: S = exp(tri), T = S^T ----
        with (
            tc.tile_pool(name="uop", bufs=1) as uop,
            tc.tile_pool(name="upsum", bufs=2, space="PSUM") as upsum,
        ):
            lhsU = uop.tile([P, 2, SEQ], bf16)
            rhsU = uop.tile([P, 2, SEQ], bf16)
            nc.sync.dma_start(lhsU.rearrange("p a b -> p (a b)"), u_lhs[:])
            nc.sync.dma_start(rhsU.rearrange("p a b -> p (a b)"), u_rhs[:])
            nc.sync.dma_start(comb.rearrange("p s c -> p (s c)"), othg[:])
            nc.sync.dma_start(ownd_sb.rearrange("p s c -> p (s c)"), ownd[:])
            for s in range(NS):
                ps = upsum.tile([P, SEQ], f32, tag="ups")
                for n4 in range(4):
                    nsl = slice(n4 * 512, (n4 + 1) * 512)
                    for k in range(2):
                        nc.tensor.matmul(
                            ps[:, nsl],
                            lhsT=lhsU[:, k, s * P : (s + 1) * P],
                            rhs=rhsU[:, k, nsl],
                            start=(k == 0),
                            stop=(k == 1),
                        )
                nc.scalar.activation(S[:, s, :], ps[:], Exp)
                # transpose strip s into column s*P of every T strip
                nc.sync.dma_start_transpose(T[:, :, s * P : (s + 1) * P], S[:, s, :])

        # ---- products ----
        with (
            tc.tile_pool(name="q1ps", bufs=2, space="PSUM") as q1ps,
            tc.tile_pool(name="p12ps", bufs=2, space="PSUM") as p12ps,
            tc.tile_pool(name="tmp", bufs=4) as tmp,
            tc.tile_pool(name="gpool", bufs=2) as gpool,
        ):
            # Q1 (oth-indexed): needs only S, overlaps the transpose DMAs.
            # gamma-scaled + normalized, written straight into comb as P2 rhs.
            for j in range(NS):
                ps = q1ps.tile([P, 257], f32, tag="q1")
                for k in range(NS):
                    nc.tensor.matmul(
                        ps[:],
                        lhsT=S[:, k, j * P : (j + 1) * P],
                        rhs=ownd_sb[:, k, 0:257],
                        start=(k == 0),
                        stop=(k == NS - 1),
                    )
                r = tmp.tile([P, 1], f32, tag="r")
                nc.vector.reciprocal(r[:], ps[:, 256:257])
                sc = tmp.tile([P, 1], f32, tag="sc")
                nc.vector.tensor_tensor(sc[:], r[:], comb[:, j, 256:257], mult)
                nc.vector.tensor_scalar_mul(
                    comb[:, j, 257:513], ps[:, 0:256], sc[:]
                )

            # P1 + sumO + P2 in one wide MM per lhsT chunk; assemble output.
            gview = g.rearrange("(G t p) c -> p G t c", p=P, t=2)
            for grp in range(NS // 2):
                gt = gpool.tile([P, 2, 5 * H], f32, tag="gt")
                for t in range(2):
                    j = grp * 2 + t
                    ps1 = p12ps.tile([P, 257], f32, tag="p1")
                    ps2 = p12ps.tile([P, H], f32, tag="p2")
                    for k in range(NS):
                        # two MMs share one lhsT chunk (LDW amortization)
                        nc.tensor.matmul(
                            ps1[:],
                            lhsT=T[:, k, j * P : (j + 1) * P],
                            rhs=comb[:, k, 0:257],
                            start=(k == 0),
                            stop=(k == NS - 1),
                        )
                        nc.tensor.matmul(
                            ps2[:],
                            lhsT=T[:, k, j * P : (j + 1) * P],
                            rhs=comb[:, k, 257:513],
                            start=(k == 0),
                            stop=(k == NS - 1),
                        )
                    r = tmp.tile([P, 1], f32, tag="r2")
                    nc.vector.reciprocal(r[:], ps1[:, 256:257])
                    nc.sync.dma_start(gt[:, t, 0:H], own_f32[j * P : (j + 1) * P, :])
                    nc.vector.tensor_scalar_mul(gt[:, t, H : 2 * H], ps1[:, 0:256], r[:])
                    nc.vector.tensor_scalar_mul(
                        gt[:, t, 2 * H : 3 * H], ps2[:], r[:]
                    )
                    nc.vector.tensor_tensor(
                        gt[:, t, 3 * H : 4 * H], gt[:, t, 0:H], gt[:, t, H : 2 * H], mult
                    )
                    nc.vector.tensor_tensor(
                        gt[:, t, 4 * H : 5 * H],
                        gt[:, t, 0:H],
                        gt[:, t, 2 * H : 3 * H],
                        mult,
                    )
                nc.sync.dma_start(gview[:, grp], gt[:])


def build_nc(reps=1):
    nc = bacc.Bacc(
        "TRN2", target_bir_lowering=False, debug=False, num_devices=N_CORES
    )
    own_f32 = nc.dram_tensor("own_f32", [SEQ, H], f32, kind="ExternalInput").ap()
    u_lhs = nc.dram_tensor("u_lhs", [P, 2 * SEQ], bf16, kind="ExternalInput").ap()
    u_rhs = nc.dram_tensor("u_rhs", [P, 2 * SEQ], bf16, kind="ExternalInput").ap()
    othg = nc.dram_tensor("othg", [P, NS * RW], bf16, kind="ExternalInput").ap()
    ownd = nc.dram_tensor("ownd", [P, NS * RC], bf16, kind="ExternalInput").ap()
    g = nc.dram_tensor("g", [SEQ, 5 * H], f32, kind="ExternalOutput").ap()
    with tile.TileContext(nc) as tc:
        for _ in range(reps):
            emit_kernel(nc, tc, own_f32, u_lhs, u_rhs, othg, ownd, g)
    nc.compile()
    return nc


def _pmajor(x, inner):
    """[K*P, C] -> [P, K*C] with partition-major swizzle for direct DMA."""
    kp, c = x.shape
    k = kp // inner
    return np.ascontiguousarray(
        x.reshape(k, inner, c).transpose(1, 0, 2).reshape(inner, k * c)
    )


def make_core_inputs(own, oth, w_own, w_oth, w_m, own_mask, oth_mask):
    """Host-side prep of one core's tensors (all small [2048,256]-ish work)."""
    own = np.asarray(own, np.float32)
    oth = np.asarray(oth, np.float32)
    own_bias = np.where(own_mask < 0.5, np.float32(-1e9), np.float32(0.0))
    oth_bias = np.where(oth_mask < 0.5, np.float32(-1e9), np.float32(0.0))
    delta = np.exp(own @ w_own + own_bias).astype(np.float32)
    gamma = np.exp(oth @ w_oth + oth_bias).astype(np.float32)

    ownm_t = np.ascontiguousarray((own * w_m[None, :]).T)  # [H, SEQ]
    oth_t = np.ascontiguousarray(oth.T)

    ownd = np.zeros((SEQ, RC), np.float32)
    ownd[:, :H] = own * delta[:, None]
    ownd[:, H] = delta
    othg = np.zeros((SEQ, RW), np.float32)
    othg[:, :H] = oth * gamma[:, None]
    othg[:, H] = gamma

    return {
        "own_f32": np.ascontiguousarray(own),
        "u_lhs": _pmajor(ownm_t, P).astype(bf16_np),
        "u_rhs": _pmajor(oth_t, P).astype(bf16_np),
        "othg": _pmajor(othg, P).astype(bf16_np),
        "ownd": _pmajor(ownd, P).astype(bf16_np),
    }


def make_all_inputs(encode_input1, encode_input2, input1_mask, input2_mask, W):
    E_q = np.asarray(encode_input1, np.float32)  # [B, SQ, H]
    E_p = np.asarray(encode_input2, np.float32)  # [B, SP, H]
    m1 = np.asarray(input1_mask, np.float32)  # [B, SP] masks p
    m2 = np.asarray(input2_mask, np.float32)  # [B, SQ] masks q
    W = np.asarray(W, np.float32)
    w_q, w_p, w_m = W[:H], W[H : 2 * H], W[2 * H :]

    in_maps = []
    for c in range(N_CORES):
        b, side = c // 2, c % 2
        if side == 0:  # produces G_q_p[b] (p-indexed)
            in_maps.append(
                make_core_inputs(E_p[b], E_q[b], w_p, w_q, w_m, m1[b], m2[b])
            )
        else:  # produces G_p_q[b] (q-indexed)
            in_maps.append(
                make_core_inputs(E_q[b], E_p[b], w_q, w_p, w_m, m2[b], m1[b])
            )
    return in_maps


_NC_CACHE = {}


def get_nc():
    if "nc" not in _NC_CACHE:
        _NC_CACHE["nc"] = build_nc()
    return _NC_CACHE["nc"]


def kernel(encode_input1, encode_input2, input1_mask, input2_mask, W):
    nc = get_nc()
    in_maps = make_all_inputs(
        encode_input1, encode_input2, input1_mask, input2_mask, W
    )
    res = run_bass_kernel_spmd(nc, in_maps, list(range(N_CORES)))
    G_q_p = np.stack([res.results[2 * b]["g"] for b in range(B)])
    G_p_q = np.stack([res.results[2 * b + 1]["g"] for b in range(B)])
    return (G_p_q, G_q_p)


if __name__ == "__main__":
    rng = np.random.default_rng(0)
    ins = {
        "encode_input1": rng.standard_normal((B, SEQ, H), np.float32),
        "encode_input2": rng.standard_normal((B, SEQ, H), np.float32),
        "input1_mask": np.ones((B, SEQ), np.float32),
        "input2_mask": np.ones((B, SEQ), np.float32),
        "W": (rng.standard_normal(3 * H) / np.sqrt(3 * H)).astype(np.float32),
    }
    out = kernel(**ins)
    print([o.shape for o in out])

